# revision 35
# baseline (speedup 1.0000x reference)
"""AlphaFold2 axial (row/column) MSA attention on 8 Trainium2 NeuronCores.

Problem: x (1, 32768, 256) = 128 MSA rows x 256 columns x dim 256.
  - width attention: softmax attention across the 128 rows, independent per
    column (256 independent length-128 sequences), 8 heads x 64.
  - height attention: "tied" attention across the 256 columns: logits are
    summed over all 128 rows, one (256x256) softmax per head shared by all
    rows.

Sharding (8 cores):
  - width: each core owns 32 columns (fully local).
  - height: each core owns 16 rows; per-core partial logits (8,256,256) are
    AllReduce'd (bf16, 2x 512KB halves) across cores, softmax replicated,
    attn*V local.

Layout strategy (everything bf16 into the PE, fp32 accumulation):
  - activations feature-major ("xT": features on partitions, tokens on free),
    prepared host-side, so projections and q.k^T need no on-device transpose.
  - scores are computed transposed, S^T = (j, i), by swapping matmul
    operands; softmax denominators are computed with an all-ones stationary
    matmul (partition-dim sum + broadcast in one PE op), normalization via
    reciprocal + multiply; no max-subtraction (logits are ~N(0, 0.1)).
  - attn*V consumes S^T directly and yields o^T feature-major, which feeds
    the output projection; outputs are written feature-major bf16 and
    transposed back on the host.

PSUM rules honored here: a matmul accumulation chain must fully finish
before another chain's start=True touches the same PSUM bank (start clears
has_written bank-wide; data values persist).
"""

import sys

for _p in ("/opt/trn_rl_repo",):
    if _p not in sys.path:
        sys.path.append(_p)

import numpy as np
import ml_dtypes

import concourse.bass as bass
import concourse.mybir as mybir
import concourse.tile as tile
from concourse import bacc
from concourse.bass_utils import run_bass_kernel_spmd

BF16 = mybir.dt.bfloat16
F32 = mybir.dt.float32
NPBF16 = ml_dtypes.bfloat16
EXP = mybir.ActivationFunctionType.Exp

N_CORES = 8
NO_COLLECTIVE = False  # timing experiment: skip AllReduce (wrong results)
PHASES = "ABC"         # timing experiment: "A" or "AB" gut later phases
H_ROWS = 128          # MSA rows
W_COLS = 256          # sequence length (columns)
D = 256               # model dim
NH = 8                # heads
DH = 64               # head dim
INNER = NH * DH       # 512
WPC = W_COLS // N_CORES   # 32 columns per core
RPC = H_ROWS // N_CORES   # 16 rows per core
T = 4096              # tokens per shard (WPC*H_ROWS == RPC*W_COLS)
SCALE = DH ** -0.5                   # 0.125
TIE_SCALE = SCALE * (H_ROWS ** -0.5)


def _ap(h):
    return h.ap()


def build_bass(loop=1):
    nc = bacc.Bacc("TRN2", target_bir_lowering=False, debug=False,
                   num_devices=N_CORES)

    # ---- per-core I/O ----
    xw = _ap(nc.dram_tensor("xw", [D, T], BF16, kind="ExternalInput"))
    xr = _ap(nc.dram_tensor("xr", [D, T], BF16, kind="ExternalInput"))
    wqk = _ap(nc.dram_tensor("wqk", [D, 2 * INNER], BF16, kind="ExternalInput"))
    wv = _ap(nc.dram_tensor("wv", [D, INNER], BF16, kind="ExternalInput"))
    wo = _ap(nc.dram_tensor("wo", [INNER, D], BF16, kind="ExternalInput"))
    hqk = _ap(nc.dram_tensor("hqk", [D, 2 * INNER], BF16, kind="ExternalInput"))
    hv = _ap(nc.dram_tensor("hv", [D, INNER], BF16, kind="ExternalInput"))
    ho = _ap(nc.dram_tensor("ho", [INNER, D], BF16, kind="ExternalInput"))
    w_out_t = _ap(nc.dram_tensor("w_out_t", [D, T], BF16, kind="ExternalOutput"))
    h_out_t = _ap(nc.dram_tensor("h_out_t", [D, T], BF16, kind="ExternalOutput"))

    with tile.TileContext(nc) as tc:
        for it in range(loop):
            # collective buffers must be distinct per unrolled iteration;
            # four quarters so the AllReduce pipeline starts after the
            # first head-pair's logits and overlaps the rest of the kernel
            cc_in = [_ap(nc.dram_tensor(f"cc_in{it}_{h}", [128, NH * 128],
                                        BF16, kind="Internal"))
                     for h in range(4)]
            cc_out = [_ap(nc.dram_tensor(f"cc_out{it}_{h}", [128, NH * 128],
                                         BF16, kind="Internal",
                                         addr_space="Shared"))
                      for h in range(4)]
            build_tile_kernel(tc, xw, xr, wqk, wv, wo, hqk, hv, ho,
                              w_out_t, h_out_t, cc_in, cc_out)
            tc.tile_update_base_wait()

    nc.compile()
    return nc


def build_tile_kernel(tc, xw, xr, wqk, wv, wo, hqk, hv, ho,
                      w_out_t, h_out_t, cc_in, cc_out):
    from contextlib import ExitStack

    nc = tc.nc
    ctx = ExitStack()

    # round-robin PSUM->SBUF copy engine (only ACT/DVE can read PSUM)
    _eng = [0]

    def copy_ps(out, in_):
        _eng[0] = (_eng[0] + 1) % 5
        if _eng[0] in (1, 3):
            nc.vector.tensor_copy(out=out, in_=in_)
        else:
            nc.scalar.copy(out=out, in_=in_)

    consts = ctx.enter_context(tc.tile_pool(name="consts", bufs=1))
    # tiles for the reduced height logits, allocated up-front so their loads
    # and exps can run as soon as the collectives land
    hdr = ctx.enter_context(tc.tile_pool(name="hdr", bufs=1))

    # ---- constants / weights into SBUF ----
    def load_wide(ap_in, name, width):  # (256, width) -> 2 chunks (128, width)
        ts = []
        for kc in range(2):
            t = consts.tile([128, width], BF16, name=f"{name}{kc}")
            nc.sync.dma_start(out=t, in_=ap_in[kc * 128:(kc + 1) * 128, :])
            ts.append(t)
        return ts

    def load_w4(ap_in, name):  # (512, 256) -> 4 chunks (128, 256)
        ts = []
        for f in range(4):
            t = consts.tile([128, D], BF16, name=f"{name}{f}")
            nc.sync.dma_start(out=t, in_=ap_in[f * 128:(f + 1) * 128, :])
            ts.append(t)
        return ts

    # phase-A dependencies first so the PE can start ASAP
    hqk_sb = load_wide(hqk, "hqk", 2 * INNER)  # [:, :512] = hq, [:, 512:] = hk
    # chunked loads so the first projection matmuls can start early
    xr_sb = []
    for kc in range(2):
        t = consts.tile([128, T], BF16, name=f"xr{kc}")
        xr_sb.append(t)
    for half in range(2):
        for kc in range(2):
            # scalar-engine DMA queue: overlaps descriptor processing with
            # the sync-queue weight loads
            nc.scalar.dma_start(
                out=xr_sb[kc][:, half * 2048:(half + 1) * 2048],
                in_=xr[kc * 128:(kc + 1) * 128,
                       half * 2048:(half + 1) * 2048])
    xw_sb = []
    for kc in range(2):
        t = consts.tile([128, T], BF16, name=f"xw{kc}")
        xw_sb.append(t)
    for half in range(2):
        for kc in range(2):
            nc.gpsimd.dma_start(
                out=xw_sb[kc][:, half * 2048:(half + 1) * 2048],
                in_=xw[kc * 128:(kc + 1) * 128,
                       half * 2048:(half + 1) * 2048])
    wqk_sb = load_wide(wqk, "wqk", 2 * INNER)  # [:, :512] = wq, [:, 512:] = wk
    wv_sb = load_wide(wv, "wv", INNER)
    hv_sb = load_wide(hv, "hv", INNER)
    wo_sb = load_w4(wo, "wo")
    ho_sb = load_w4(ho, "ho")

    ones_sb = consts.tile([128, 128], BF16, name="ones")
    nc.vector.memset(ones_sb, 1.0)

    # ---------------------------------------------------------------
    # Phase A: height q/k projections + partial tied logits; AllReduce.
    # dots^T[H](j, i) = sum_r sum_d k[r,j,H,d] q[r,i,H,d]  (j,i = columns)
    # Interleaved per head-pair f so each half of the logits can enter its
    # AllReduce as soon as it is ready.
    # ---------------------------------------------------------------
    dotsr = [hdr.tile([128, NH * 128], BF16, name=f"dotsr{h}")
             for h in range(4)]
    Eh = hdr.tile([128, NH * 512], BF16, name="Eh")
    BinvH2 = hdr.tile([128, 4 * 256], F32, name="BinvH2")

    # psDA bufs=1: the next head-pair's projections (3.4us of PE work) fully
    # cover the copy drain, and phase A then needs only 6 PSUM banks -- so in
    # the steady-state loop it can start while the previous iteration's
    # output projection (banks 6-7) is still draining.
    with tc.tile_pool(name="phaseA", bufs=1) as phaseA, \
         tc.tile_pool(name="dotsA", bufs=1) as dotsA, \
         tc.tile_pool(name="psA", bufs=2, space="PSUM") as psA, \
         tc.tile_pool(name="psDA", bufs=1, space="PSUM") as psDA:

        dots_sb = dotsA.tile([128, NH * 512], BF16, name="dots_sb")

        def proj_chunk(w_off, f, name):
            """(feat, tok) chunk f of x_r @ w: (128, T) bf16, feature-major.
            Two nt halves share one (128,1024) PSUM tile -> one wide copy."""
            t = phaseA.tile([128, T], BF16, name=f"{name}{f}")
            for np_ in range(T // 1024):
                ps = psA.tile([128, 1024], F32, tag="projA", name="projA")
                for sub in range(2):
                    nt = np_ * 2 + sub
                    for kc in range(2):
                        nc.tensor.matmul(
                            out=ps[:, sub * 512:(sub + 1) * 512],
                            lhsT=hqk_sb[kc][:, w_off + f * 128:
                                            w_off + (f + 1) * 128],
                            rhs=xr_sb[kc][:, nt * 512:(nt + 1) * 512],
                            start=(kc == 0), stop=(kc == 1))
                copy_ps(t[:, np_ * 1024:(np_ + 1) * 1024], ps)
            return t

        for f in range(4):
            qf = proj_chunk(0, f, "qhT")
            kf = proj_chunk(INNER, f, "khT")
            # heads 2f (free 0:512, bank 0) and 2f+1 (free 512:1024, bank 1)
            dps = psDA.tile([128, 1024], F32, tag="hdots", name="hdots")
            # complete each jc accumulation chain before the next starts in
            # the same bank (start=True clears has_written bank-wide)
            for jc in range(2):
                for r in range(RPC):
                    for hp in range(2):
                        b = hp * 64
                        nc.tensor.matmul(
                            out=dps[:, hp * 512 + jc * 256:
                                    hp * 512 + (jc + 1) * 256],
                            lhsT=kf[b:b + 64, r * 256 + jc * 128:
                                    r * 256 + jc * 128 + 128],
                            rhs=qf[b:b + 64, r * 256:(r + 1) * 256],
                            start=(r == 0), stop=(r == RPC - 1))
            copy_ps(dots_sb[:, 2 * f * 512:(2 * f + 2) * 512], dps)
            nc.sync.dma_start(
                out=cc_in[f][:, :],
                in_=dots_sb[:, f * 1024:(f + 1) * 1024])
            if NO_COLLECTIVE:
                nc.gpsimd.dma_start(out=dotsr[f][:, :],
                                    in_=cc_in[f][:, :])
            else:
                nc.gpsimd.collective_compute(
                    "AllReduce", mybir.AluOpType.add,
                    replica_groups=[list(range(N_CORES))],
                    ins=[cc_in[f].opt()], outs=[cc_out[f].opt()])
                # early read-back on the gpsimd queue (sits behind the
                # collective; cannot block the sync-queue stores)
                nc.gpsimd.dma_start(out=dotsr[f][:, :],
                                    in_=cc_out[f][:, :])

    if PHASES == "A":
        with tc.tile_pool(name="zf", bufs=1) as zf:
            z = zf.tile([128, 1024], BF16, name="z")
            nc.vector.memset(z, 0.0)
            for mc in range(2):
                for np_ in range(T // 1024):
                    nc.sync.dma_start(
                        out=w_out_t[mc * 128:(mc + 1) * 128,
                                    np_ * 1024:(np_ + 1) * 1024], in_=z)
                    nc.sync.dma_start(
                        out=h_out_t[mc * 128:(mc + 1) * 128,
                                    np_ * 1024:(np_ + 1) * 1024], in_=z)
        ctx.close()
        return

    # ---------------------------------------------------------------
    # Phase B: width attention over this core's 32 columns.
    # Ew slot layout: slot(H) = (H%2)*512 + (H//2)*128
    # ---------------------------------------------------------------
    NCG = 8                      # columns per group
    NGRP = WPC // NCG            # 4 groups
    GT = NCG * 128               # tokens per group (1024)

    with tc.tile_pool(name="phaseB", bufs=1) as phaseB, \
         tc.tile_pool(name="grpB", bufs=2) as grpB, \
         tc.tile_pool(name="colB", bufs=6) as colB, \
         tc.tile_pool(name="psW", bufs=2, space="PSUM") as psW, \
         tc.tile_pool(name="psBS", bufs=2, space="PSUM") as psBS, \
         tc.tile_pool(name="psSO", bufs=2, space="PSUM") as psSO:
        psB = psSO

        # o^T accumulator: (128, f, tok) -- chunk f holds heads 2f, 2f+1
        owT = phaseB.tile([128, 4, T], BF16, name="owT")

        def emit_group_proj_chunks(g):
            """Return a list of thunks, each emitting one projection chunk
            for group g (4 matmuls into a shared wide PSUM + 1 wide copy)."""
            tok0 = g * GT
            qwT, kwT, vw = [], [], []
            thunks = []
            for f in range(4):
                for which, lst in ((0, qwT), (1, kwT)):
                    w_off = which * INNER
                    t = grpB.tile([128, GT], BF16, tag=f"qk{which}{f}",
                                  name=f"qk{which}{f}")
                    lst.append(t)
                    for nt in range(GT // 512):
                        def th(w_off=w_off, t=t, nt=nt, f=f, tok0=tok0):
                            ps = psB.tile([128, 512], F32, tag="so512",
                                          name="projB")
                            for kc in range(2):
                                nc.tensor.matmul(
                                    out=ps,
                                    lhsT=wqk_sb[kc][:, w_off + f * 128:
                                                    w_off + (f + 1) * 128],
                                    rhs=xw_sb[kc][:, tok0 + nt * 512:
                                                  tok0 + (nt + 1) * 512],
                                    start=(kc == 0), stop=(kc == 1))
                            copy_ps(t[:, nt * 512:(nt + 1) * 512], ps)
                        thunks.append(th)
            for ci in range(NCG):
                t = grpB.tile([128, INNER], BF16, tag=f"vw{ci}",
                              name=f"vw{ci}")
                vw.append(t)
                def th(t=t, ci=ci, tok0=tok0):
                    ps = psB.tile([128, 512], F32, tag="so512", name="projB")
                    for kc in range(2):
                        nc.tensor.matmul(
                            out=ps,
                            lhsT=xw_sb[kc][:, tok0 + ci * 128:
                                           tok0 + (ci + 1) * 128],
                            rhs=wv_sb[kc],
                            start=(kc == 0), stop=(kc == 1))
                    copy_ps(t, ps)
                thunks.append(th)
            return (qwT, kwT, vw), thunks

        def emit_col_dots_exp(tiles, g, ci):
            qwT, kwT, vw = tiles
            c0 = ci * 128  # token offset within group
            # scores^T in one wide PSUM tile: [:, hp*512 + f*128 + i]
            dps = psW.tile([128, 1024], F32, tag="wdots", name="wdots")
            for f in range(4):
                for hp in range(2):
                    b = hp * 64
                    nc.tensor.matmul(
                        out=dps[:, hp * 512 + f * 128:
                                hp * 512 + (f + 1) * 128],
                        lhsT=kwT[f][b:b + 64, c0:c0 + 128],
                        rhs=qwT[f][b:b + 64, c0:c0 + 128],
                        start=True, stop=True)
            Ew = colB.tile([128, 1024], BF16, tag="Ew", name="Ew")
            # single wide exp for both hp halves
            nc.scalar.activation(out=Ew, in_=dps, func=EXP, scale=SCALE)
            return Ew

        def emit_col_av(tiles, g, ci, Ew):
            qwT, kwT, vw = tiles
            tok0 = g * GT
            c0 = ci * 128
            Binv2 = colB.tile([128, 512], F32, tag="Binv2", name="Binv2")
            ops = psSO.tile([128, 512], F32, tag="so512", name="opsW")
            # bsum2 partition-halves: 0:64 <- hp0 sums, 64:128 <- hp1 sums
            # (M=64 col-tiled ones-matmuls; B rows are identical so the
            # half-height broadcast carries the right values)
            bsum2 = psBS.tile([128, 512], F32, tag="bsum2", name="bsum2")
            for hp in range(2):
                hb = hp * 512
                p0 = hp * 64
                # attn*V on UNNORMALIZED E -- normalization is deferred to
                # the PSUM->SBUF o^T copy; the exp is the only thing the
                # AV matmuls wait for.
                for f in range(4):
                    H = 2 * f + hp
                    nc.tensor.matmul(
                        out=ops[hp * 64:hp * 64 + 64,
                                f * 128:(f + 1) * 128],
                        lhsT=vw[ci][:, H * 64:(H + 1) * 64],
                        rhs=Ew[:, hb + f * 128:hb + (f + 1) * 128],
                        start=True, stop=True)
                nc.tensor.matmul(out=bsum2[p0:p0 + 64, :],
                                 lhsT=ones_sb[:, 0:64],
                                 rhs=Ew[:, hb:hb + 512],
                                 start=True, stop=True)
            nc.vector.reciprocal_approx_fast(out=Binv2, in_=bsum2)
            # o^T = o_unnorm * 1/s, fused with the PSUM->SBUF o^T copy
            nc.vector.tensor_mul(
                out=owT[:, :, tok0 + c0:tok0 + c0 + 128],
                in0=ops.rearrange("p (f i) -> p f i", f=4),
                in1=Binv2.rearrange("p (f i) -> p f i", f=4))

        # software pipeline, one column deep: column c+1's dots run on the
        # PE while column c's exp finishes on ScalarE, and group g+1's
        # projections interleave so the PE always has independent work
        all_cols = [(g, ci) for g in range(NGRP) for ci in range(NCG)]
        tiles_by_g = {0: None}
        tiles_by_g[0], thunks0 = emit_group_proj_chunks(0)
        for th in thunks0:
            th()
        nxt = None
        per_col = 0
        pending = None
        for idx, (g, ci) in enumerate(all_cols):
            if ci == 0 and g + 1 < NGRP:
                tiles_by_g[g + 1], nxt_thunks = emit_group_proj_chunks(g + 1)
                nxt = iter(nxt_thunks)
                per_col = (len(nxt_thunks) + NCG - 1) // NCG
            Ew = emit_col_dots_exp(tiles_by_g[g], g, ci)
            if pending is not None:
                emit_col_av(*pending)
            pending = (tiles_by_g[g], g, ci, Ew)
            if nxt is not None:
                for _ in range(per_col):
                    th = next(nxt, None)
                    if th is not None:
                        th()
            if ci == NCG - 1 and nxt is not None:
                for th in nxt:
                    th()
                nxt = None
        emit_col_av(*pending)

        # width output projection: w_out^T = wo^T @ o^T
        with tc.tile_pool(name="stgB", bufs=3) as stgB:
            for mc in range(2):
                for np_ in range(T // 2048):
                    st = stgB.tile([128, 2048], BF16, tag="stgW", name="stgW")
                    for sub in range(4):
                        nt = np_ * 4 + sub
                        ps = psSO.tile([128, 512], F32, tag="so512",
                                       name="oprojW")
                        for f in range(4):
                            nc.tensor.matmul(
                                out=ps,
                                lhsT=wo_sb[f][:, mc * 128:(mc + 1) * 128],
                                rhs=owT[:, f, nt * 512:(nt + 1) * 512],
                                start=(f == 0), stop=(f == 3))
                        copy_ps(st[:, sub * 512:(sub + 1) * 512], ps)
                    nc.sync.dma_start(
                        out=w_out_t[mc * 128:(mc + 1) * 128,
                                    np_ * 2048:(np_ + 1) * 2048],
                        in_=st)

    if PHASES == "AB":
        with tc.tile_pool(name="zf", bufs=1) as zf:
            z = zf.tile([128, 1024], BF16, name="z")
            nc.vector.memset(z, 0.0)
            for mc in range(2):
                for np_ in range(T // 1024):
                    nc.sync.dma_start(
                        out=h_out_t[mc * 128:(mc + 1) * 128,
                                    np_ * 1024:(np_ + 1) * 1024], in_=z)
        ctx.close()
        return

    # ---------------------------------------------------------------
    # Phase C: height attention finish (after AllReduce).
    # ---------------------------------------------------------------
    with tc.tile_pool(name="phaseC", bufs=1) as phaseC, \
         tc.tile_pool(name="stgC", bufs=3) as stgC:

        # exponentiate the reduced height logits (ScalarE) while the PE
        # runs the v projections below.  tile_wait_until pins these late in
        # the scheduler's model so no phase-B work can queue up behind a
        # wait on the collective (head-of-line blocking).
        with tc.tile_wait_until(ms=0.2):
            for H in range(NH):
                nc.scalar.activation(
                    out=Eh[:, H * 512:(H + 1) * 512],
                    in_=dotsr[H // 2][:, (H % 2) * 512:(H % 2 + 1) * 512],
                    func=EXP, scale=TIE_SCALE)

        # v (token-major) for the row shard: 16 chunks (128, 1024) covering
        # two 128-token groups each; independent of the collective.
        vh = []
        with tc.tile_pool(name="psV", bufs=2, space="PSUM") as psV:
            for rc2 in range(16):
                t = phaseC.tile([128, 2 * INNER], BF16, name=f"vh{rc2}")
                vh.append(t)
                ps = psV.tile([128, 1024], F32, tag="projV", name="projV")
                for sub in range(2):
                    rc = rc2 * 2 + sub
                    for kc in range(2):
                        nc.tensor.matmul(
                            out=ps[:, sub * 512:(sub + 1) * 512],
                            lhsT=xr_sb[kc][:, rc * 128:(rc + 1) * 128],
                            rhs=hv_sb[kc],
                            start=(kc == 0), stop=(kc == 1))
                copy_ps(t, ps)

        # denominators: B_H(i) = sum over both j-chunks and partitions.
        # BinvH2[p, f*256+i] = 1/s_{2f + (p>=64)}(i): parity-split partition
        # halves so the deferred normalize-mul below runs full-width.
        with tc.tile_pool(name="psSC", bufs=2, space="PSUM") as psSC:
            for f in range(4):
                bps = psSC.tile([128, 256], F32, tag="bsumH", name="bsumH")
                for hp in range(2):
                    H = 2 * f + hp
                    p0 = hp * 64
                    # complete each hp chain before the next (same bank)
                    for jc in range(2):
                        nc.tensor.matmul(
                            out=bps[p0:p0 + 64, :],
                            lhsT=ones_sb[:, 0:64],
                            rhs=Eh[:, H * 512 + jc * 256:
                                   H * 512 + (jc + 1) * 256],
                            start=(jc == 0), stop=(jc == 1))
                nc.vector.reciprocal_approx_fast(
                    out=BinvH2[:, f * 256:(f + 1) * 256], in_=bps)

        # attn * V per row -> o^T chunks; ohT free = (f, r*256 + i).
        # All AV groups run back-to-back (normalize-muls trail on DVE);
        # the output projections follow densely afterwards.
        ohT = phaseC.tile([128, 4, T], BF16, name="ohT")
        with tc.tile_pool(name="psOC", bufs=3, space="PSUM") as psOC:
            for r in range(RPC):
                ops = psOC.tile([128, 1024], F32, tag="opsH", name="opsH")
                for f in range(4):
                    for hp in range(2):
                        H = 2 * f + hp
                        for jc in range(2):
                            vt = vh[r]
                            vo = jc * INNER
                            nc.tensor.matmul(
                                out=ops[hp * 64:hp * 64 + 64,
                                        f * 256:(f + 1) * 256],
                                lhsT=vt[:, vo + H * 64:vo + (H + 1) * 64],
                                rhs=Eh[:, H * 512 + jc * 256:
                                       H * 512 + (jc + 1) * 256],
                                start=(jc == 0), stop=(jc == 1))
                nc.vector.tensor_mul(
                    out=ohT[:, :, r * 256:(r + 1) * 256],
                    in0=ops.rearrange("p (f i) -> p f i", f=4),
                    in1=BinvH2.rearrange("p (f i) -> p f i", f=4))

            # opened while psOC still holds banks 0-5 so this long-lived pool
            # lands on banks 6-7: the next loop iteration's phase A (6 banks)
            # then only waits on the AV matmuls, not on this drain
            with tc.tile_pool(name="psO2", bufs=2, space="PSUM") as psO2:
                for np_ in range(RPC // 8):
                    for mc in range(2):
                        st = stgC.tile([128, 2048], BF16, tag="stgH",
                                       name="stgH")
                        for sub in range(4):
                            nt = np_ * 4 + sub
                            ps = psO2.tile([128, 512], F32, tag="projC",
                                           name="oprojH")
                            for f in range(4):
                                nc.tensor.matmul(
                                    out=ps,
                                    lhsT=ho_sb[f][:, mc * 128:(mc + 1) * 128],
                                    rhs=ohT[:, f, nt * 512:(nt + 1) * 512],
                                    start=(f == 0), stop=(f == 3))
                            copy_ps(st[:, sub * 512:(sub + 1) * 512], ps)
                        nc.sync.dma_start(
                            out=h_out_t[mc * 128:(mc + 1) * 128,
                                        np_ * 2048:(np_ + 1) * 2048],
                            in_=st)

    ctx.close()


_NC = None


def _get_nc():
    global _NC
    if _NC is None:
        _NC = build_bass()
    return _NC


def make_in_maps(x, wq_w, wkv_w, wout_w, hq_w, hkv_w, hout_w):
    x4 = np.asarray(x, np.float32).reshape(H_ROWS, W_COLS, D)
    xb = x4.astype(NPBF16)
    wkv = np.asarray(wkv_w, np.float32)
    hkv = np.asarray(hkv_w, np.float32)
    wqk = np.concatenate([np.asarray(wq_w, np.float32), wkv[:, :INNER]],
                         axis=1)
    hqk = np.concatenate([np.asarray(hq_w, np.float32), hkv[:, :INNER]],
                         axis=1)
    wghts = {
        "wqk": np.ascontiguousarray(wqk.astype(NPBF16)),
        "wv": np.ascontiguousarray(wkv[:, INNER:].astype(NPBF16)),
        "wo": np.ascontiguousarray(np.asarray(wout_w, np.float32).astype(NPBF16)),
        "hqk": np.ascontiguousarray(hqk.astype(NPBF16)),
        "hv": np.ascontiguousarray(hkv[:, INNER:].astype(NPBF16)),
        "ho": np.ascontiguousarray(np.asarray(hout_w, np.float32).astype(NPBF16)),
    }
    in_maps = []
    for c in range(N_CORES):
        xw_c = np.ascontiguousarray(
            xb[:, c * WPC:(c + 1) * WPC, :].transpose(1, 0, 2)
            .reshape(T, D).T)
        xr_c = np.ascontiguousarray(xb[c * RPC:(c + 1) * RPC].reshape(T, D).T)
        m = {"xw": xw_c, "xr": xr_c}
        m.update(wghts)
        in_maps.append(m)
    return in_maps


def assemble_output(results, wout_b, hout_b):
    w_full = np.empty((H_ROWS, W_COLS, D), np.float32)
    h_full = np.empty((H_ROWS, W_COLS, D), np.float32)
    for c in range(N_CORES):
        wt = np.asarray(results[c]["w_out_t"], dtype=np.float32)  # (256, 4096)
        w_full[:, c * WPC:(c + 1) * WPC, :] = \
            wt.T.reshape(WPC, H_ROWS, D).transpose(1, 0, 2)
        ht = np.asarray(results[c]["h_out_t"], dtype=np.float32)
        h_full[c * RPC:(c + 1) * RPC] = ht.T.reshape(RPC, W_COLS, D)
    out = w_full + h_full
    out += (np.asarray(wout_b, np.float32) + np.asarray(hout_b, np.float32))
    return out.reshape(1, H_ROWS * W_COLS, D)


def kernel(x, wq_w, wkv_w, wout_w, wout_b, hq_w, hkv_w, hout_w, hout_b,
           msa_h=H_ROWS, msa_w=W_COLS, **_unused):
    in_maps = make_in_maps(x, wq_w, wkv_w, wout_w, hq_w, hkv_w, hout_w)
    nc = _get_nc()
    res = run_bass_kernel_spmd(nc, in_maps, core_ids=list(range(N_CORES)))
    return assemble_output(res.results, wout_b, hout_b)
